# revision 11
# baseline (speedup 1.0000x reference)
"""ColorLoss (3D color histogram + L1) Trainium2 kernel — v2.

Strategy (data-parallel over batch, 8 cores):
  - Core i processes image i ([3,1024,1024]) plus 1/8 of the style image
    ([3,128,1024] row-slice).  4096 bins = 64x64 via key1 = 4*h1+rl,
    key2 = 4*b+gh with h1 = rh + 4*gl, rh=r>>2, rl=r&3, gl=g&3, gh=g>>2.
  - Encodings are EXACT fp8(e4m3) one-hots packed 4-planes-per-int32:
    word_e = (h==e) * (56 << 8*lo), byte pattern 0x38 = fp8 1.0 at byte lo.
    16 scalar_tensor_tensor ops per side per chunk write all 64 planes.
  - Per-pixel floors (r, g, b, r>>2, g>>2) computed on the Scalar engine as
    activation Copy with bias=-0.5 and int output (round -> floor).
  - Integer helper chains (rl, gl, h1, 56*256^lo) run on GPSIMD tensor_tensor
    + a few DVE tensor_scalar ops, keeping DVE mostly on the 32 stt plane ops.
  - Matmul: fp8 DoubleRow, 512 px/instr: out[m=(c,key1), n=(c',key2)] over
    [K=128 partitions x 2 slabs]; 2 c-blocks diagonal; off-diagonal junk
    discarded on host.  PSUM [128,128] f32, counts exact.
"""
import sys

sys.path.insert(0, "/opt/trn_rl_repo")
import os
import numpy as np
from contextlib import ExitStack

import ml_dtypes  # noqa: F401

# ---------------- tunables ----------------
T = 512             # pixels per partition per chunk (multiple of 4)
H, W = 1024, 1024
HW = H * W
IMG_PP = HW // 128          # 8192 pixels/partition for one image
STY_PP = 128 * W // 128     # 1024 pixels/partition for the style slice

_cache = {}


def _build():
    import concourse.bacc as bacc
    import concourse.mybir as mybir
    from concourse.tile import TileContext

    F32 = mybir.dt.float32
    I32 = mybir.dt.int32
    I16 = mybir.dt.int16
    FP8 = mybir.dt.float8e4
    Alu = mybir.AluOpType
    Act = mybir.ActivationFunctionType
    DR = mybir.MatmulPerfMode.DoubleRow

    Q = T // 4

    nc = bacc.Bacc("TRN2")
    img_d = nc.dram_tensor("img", [3, H, W], F32, kind="ExternalInput")
    sty_d = nc.dram_tensor("sty", [3, 128, W], F32, kind="ExternalInput")
    o_d = nc.dram_tensor("out", [2, 128, 128], F32, kind="ExternalOutput")

    img_v = [img_d[c, :, :].rearrange("(p r) w -> p (r w)", p=128) for c in range(3)]
    sty_v = [sty_d[c, :, :] for c in range(3)]

    with TileContext(nc) as tc:
        with ExitStack() as ctx:
            xpool = ctx.enter_context(tc.tile_pool(name="x", bufs=2))
            fpool = ctx.enter_context(tc.tile_pool(name="f", bufs=2))
            bpool = ctx.enter_context(tc.tile_pool(name="b", bufs=2))
            kpool = ctx.enter_context(tc.tile_pool(name="k", bufs=2))
            jpool = ctx.enter_context(tc.tile_pool(name="j", bufs=1))
            epool = ctx.enter_context(tc.tile_pool(name="e", bufs=2))
            cpool = ctx.enter_context(tc.tile_pool(name="c", bufs=1))
            opool = ctx.enter_context(tc.tile_pool(name="o", bufs=1))
            pspool = ctx.enter_context(tc.tile_pool(name="ps", bufs=2, space="PSUM"))

            # constants
            bcl1 = cpool.tile([128, 1], F32, tag="bcl1")
            nc.vector.memset(bcl1[:], 7.4)
            bcl2 = cpool.tile([128, 1], F32, tag="bcl2")
            nc.vector.memset(bcl2[:], 15.4)
            c56 = cpool.tile([128, 1], I32, tag="c56")
            nc.vector.memset(c56[:], 56)

            ps_img = pspool.tile([128, 128], F32)
            ps_sty = pspool.tile([128, 128], F32)

            def do_chunk(views, off, ps, start, stop, tcw=T):
                Qc = tcw // 4
                xt = xpool.tile([128, 3, T], F32, tag="xt")
                for c in range(3):
                    nc.sync.dma_start(xt[:, c, :tcw], views[c][:, off : off + tcw])

                # ACT: t = clamp(8x+8, 0, 15.4)  (u = Relu(-8x+7.4), in-place)
                tt = fpool.tile([128, 3, T], F32, tag="tt")
                for c in range(3):
                    nc.scalar.activation(tt[:, c, :tcw], xt[:, c, :tcw], Act.Relu,
                                         bias=bcl1[:], scale=-8.0)
                    nc.scalar.activation(tt[:, c, :tcw], tt[:, c, :tcw], Act.Relu,
                                         bias=bcl2[:], scale=-1.0)

                # ACT floors: r,g,rh,gh,h2=b (all i32)
                bins = bpool.tile([128, 2, T], I32, tag="bins")    # r, g
                bins4 = bpool.tile([128, 2, T], I32, tag="bins4")  # rh, gh
                h2 = kpool.tile([128, T], I32, tag="h2")           # b bin
                nc.scalar.activation(bins[:, 0, :tcw], tt[:, 0, :tcw], Act.Copy,
                                     bias=-0.5, scale=1.0)
                nc.scalar.activation(bins[:, 1, :tcw], tt[:, 1, :tcw], Act.Copy,
                                     bias=-0.5, scale=1.0)
                nc.scalar.activation(bins4[:, 0, :tcw], tt[:, 0, :tcw], Act.Copy,
                                     bias=-0.5, scale=0.25)
                nc.scalar.activation(bins4[:, 1, :tcw], tt[:, 1, :tcw], Act.Copy,
                                     bias=-0.5, scale=0.25)
                nc.scalar.activation(h2[:, :tcw], tt[:, 2, :tcw], Act.Copy,
                                     bias=-0.5, scale=1.0)

                r = bins[:, 0, :tcw]
                g = bins[:, 1, :tcw]
                rh = bins4[:, 0, :tcw]
                gh = bins4[:, 1, :tcw]

                # Pool (GPSIMD) integer chains: rl = r-4rh, gl = g-4gh,
                # h1 = rh+4gl  (all i32, via adds/subtract)
                pi = kpool.tile([128, 4, T], I32, tag="pi")
                # pi rows: 0=scratch 1=rl 2=gl 3=h1
                nc.gpsimd.tensor_tensor(pi[:, 0, :tcw], rh, rh, Alu.add)      # 2rh
                nc.gpsimd.tensor_tensor(pi[:, 0, :tcw], pi[:, 0, :tcw],
                                        pi[:, 0, :tcw], Alu.add)               # 4rh
                nc.gpsimd.tensor_tensor(pi[:, 1, :tcw], r, pi[:, 0, :tcw],
                                        Alu.subtract)                          # rl
                nc.gpsimd.tensor_tensor(pi[:, 0, :tcw], gh, gh, Alu.add)       # 2gh
                nc.gpsimd.tensor_tensor(pi[:, 0, :tcw], pi[:, 0, :tcw],
                                        pi[:, 0, :tcw], Alu.add)               # 4gh
                nc.gpsimd.tensor_tensor(pi[:, 2, :tcw], g, pi[:, 0, :tcw],
                                        Alu.subtract)                          # gl
                nc.gpsimd.tensor_tensor(pi[:, 0, :tcw], pi[:, 2, :tcw],
                                        pi[:, 2, :tcw], Alu.add)               # 2gl
                nc.gpsimd.tensor_tensor(pi[:, 0, :tcw], pi[:, 0, :tcw],
                                        pi[:, 0, :tcw], Alu.add)               # 4gl
                nc.gpsimd.tensor_tensor(pi[:, 3, :tcw], rh, pi[:, 0, :tcw],
                                        Alu.add)                               # h1

                rl = pi[:, 1, :tcw]
                gl = pi[:, 2, :tcw]
                h1 = pi[:, 3, :tcw]

                # t = 56 << (8*lo): s8 on ACT (Copy scale=8), shift on DVE
                s8 = jpool.tile([128, 2, T], I32, tag="s8")
                t1 = kpool.tile([128, T], I32, tag="t1")
                t2 = kpool.tile([128, T], I32, tag="t2")
                c56b = c56[:].broadcast_to([128, tcw])
                nc.scalar.activation(s8[:, 0, :tcw], rl, Act.Copy,
                                     bias=0.0, scale=8.0)
                nc.scalar.activation(s8[:, 1, :tcw], gh, Act.Copy,
                                     bias=0.0, scale=8.0)
                nc.vector.tensor_tensor(t1[:, :tcw], c56b, s8[:, 0, :tcw],
                                        Alu.logical_shift_left)
                nc.vector.tensor_tensor(t2[:, :tcw], c56b, s8[:, 1, :tcw],
                                        Alu.logical_shift_left)

                # DVE: packed planes, pixel t=(q,s,c)
                h1v = h1.rearrange("p (q s c) -> p q s c", s=2, c=2)
                t1v = t1[:, :tcw].rearrange("p (q s c) -> p q s c", s=2, c=2)
                e1p = epool.tile([128, Q, 2, 2, 16], I32, tag="e1p")
                e2c = epool.tile([128, 16, T], I32, tag="e2c")
                for e in range(16):
                    nc.vector.scalar_tensor_tensor(e1p[:, :Qc, :, :, e], h1v, e,
                                                   t1v, Alu.is_equal, Alu.mult)
                    nc.vector.scalar_tensor_tensor(e2c[:, e, :tcw], h2[:, :tcw],
                                                   e, t2[:, :tcw],
                                                   Alu.is_equal, Alu.mult)

                e1f = e1p[:, :, :, :, :].bitcast(FP8).rearrange(
                    "p q s c eb -> p q s (c eb)")
                e2f = e2c[:, :, :tcw].bitcast(FP8).rearrange(
                    "p e (q s cb) -> p q s e cb", s=2, cb=8)
                for q in range(Qc):
                    nc.tensor.matmul(ps[:], e1f[:, q, :, :], e2f[:, q, :, :, :],
                                     start=(start and q == 0),
                                     stop=(stop and q == Qc - 1),
                                     perf_mode=DR)

            img_sizes = [64, 128, 256] + [512] * 15 + [64]
            assert sum(img_sizes) == IMG_PP
            off = 0
            for ci, tcw in enumerate(img_sizes):
                do_chunk(img_v, off, ps_img, ci == 0,
                         ci == len(img_sizes) - 1, tcw)
                off += tcw

            # ship the image histogram while the style chunks run
            ostage = opool.tile([128, 2, 128], F32)
            nc.vector.tensor_copy(ostage[:, 0, :], ps_img[:])
            nc.sync.dma_start(o_d[0, :, :], ostage[:, 0, :])

            sty_sizes = [512, 448, 64]
            assert sum(sty_sizes) == STY_PP
            off = 0
            for ci, tcw in enumerate(sty_sizes):
                do_chunk(sty_v, off, ps_sty, ci == 0,
                         ci == len(sty_sizes) - 1, tcw)
                off += tcw

            nc.vector.tensor_copy(ostage[:, 1, :], ps_sty[:])
            nc.sync.dma_start(o_d[1, :, :], ostage[:, 1, :])

    nc.finalize()
    return nc


def _get_built():
    if "nc" not in _cache:
        _cache["nc"] = _build()
    return _cache["nc"]


def _perm():
    """flat[key1, key2] -> flat bin index (r + 16g + 256b)."""
    if "perm" in _cache:
        return _cache["perm"]
    k1 = np.arange(64)
    k2 = np.arange(64)
    rl = k1 & 3
    h1 = k1 >> 2
    rh = h1 & 3
    gl = h1 >> 2
    r = 4 * rh + rl            # [64]
    gh = k2 & 3
    b = k2 >> 2
    flat = (r[:, None] + 16 * (gl[:, None] + 4 * gh[None, :])
            + 256 * b[None, :])  # [64,64]
    _cache["perm"] = flat
    return flat


def _col_idx():
    if "colidx" not in _cache:
        k2 = np.arange(64)
        _cache["colidx"] = [(k2 // 4) * 8 + c * 4 + (k2 % 4) for c in range(2)]
    return _cache["colidx"]


def _decode(raw):
    """raw [2,128,128] f32 -> (hist_img[4096], hist_sty[4096]) exact counts."""
    flat = _perm()
    n0, n1 = _col_idx()
    out = []
    for s_ in range(2):
        m = raw[s_]
        counts64 = m[0:64, :][:, n0] + m[64:128, :][:, n1]   # [key1, key2]
        h = np.zeros(4096)
        np.add.at(h, flat.reshape(-1), counts64.reshape(-1))
        out.append(h)
    return out


def kernel(input, style_image, n_bins):
    assert int(n_bins) == 16
    from concourse import bass_utils

    nc = _get_built()
    input = np.ascontiguousarray(np.asarray(input, dtype=np.float32))
    style = np.ascontiguousarray(np.asarray(style_image, dtype=np.float32))
    B = input.shape[0]
    assert B == 8 and input.shape == (8, 3, H, W)
    in_maps = [
        {
            "img": input[i],
            "sty": np.ascontiguousarray(style[0, :, 128 * i : 128 * (i + 1), :]),
        }
        for i in range(8)
    ]
    res = bass_utils.run_bass_kernel_spmd(nc, in_maps, core_ids=list(range(8)),
                                          **_cache.get("run_kwargs", {}))
    _cache["last_results"] = res
    hists = np.zeros((B, 4096))
    sty_hist = np.zeros(4096)
    for i in range(8):
        hi, hs = _decode(res.results[i]["out"])
        hists[i] = hi
        sty_hist += hs
    cols = (hists / HW).astype(np.float32)
    target = (sty_hist / HW).astype(np.float32)
    loss = np.mean(np.abs(cols - target[None, :]).astype(np.float32))
    return np.float32(loss)


# revision 12
# speedup vs baseline: 1.0058x; 1.0058x over previous
"""ColorLoss (3D color histogram + L1) Trainium2 kernel — v2.

Strategy (data-parallel over batch, 8 cores):
  - Core i processes image i ([3,1024,1024]) plus 1/8 of the style image
    ([3,128,1024] row-slice).  4096 bins = 64x64 via key1 = 4*h1+rl,
    key2 = 4*b+gh with h1 = rh + 4*gl, rh=r>>2, rl=r&3, gl=g&3, gh=g>>2.
  - Encodings are EXACT fp8(e4m3) one-hots packed 4-planes-per-int32:
    word_e = (h==e) * (56 << 8*lo), byte pattern 0x38 = fp8 1.0 at byte lo.
    16 scalar_tensor_tensor ops per side per chunk write all 64 planes.
  - Per-pixel floors (r, g, b, r>>2, g>>2) computed on the Scalar engine as
    activation Copy with bias=-0.5 and int output (round -> floor).
  - Integer helper chains (rl, gl, h1, 56*256^lo) run on GPSIMD tensor_tensor
    + a few DVE tensor_scalar ops, keeping DVE mostly on the 32 stt plane ops.
  - Matmul: fp8 DoubleRow, 512 px/instr: out[m=(c,key1), n=(c',key2)] over
    [K=128 partitions x 2 slabs]; 2 c-blocks diagonal; off-diagonal junk
    discarded on host.  PSUM [128,128] f32, counts exact.
"""
import sys

sys.path.insert(0, "/opt/trn_rl_repo")
import os
import numpy as np
from contextlib import ExitStack

import ml_dtypes  # noqa: F401

# ---------------- tunables ----------------
T = 512             # pixels per partition per chunk (multiple of 4)
H, W = 1024, 1024
HW = H * W
IMG_PP = HW // 128          # 8192 pixels/partition for one image
STY_PP = 128 * W // 128     # 1024 pixels/partition for the style slice

_cache = {}


def _build():
    import concourse.bacc as bacc
    import concourse.mybir as mybir
    from concourse.tile import TileContext

    F32 = mybir.dt.float32
    I32 = mybir.dt.int32
    I16 = mybir.dt.int16
    FP8 = mybir.dt.float8e4
    Alu = mybir.AluOpType
    Act = mybir.ActivationFunctionType
    DR = mybir.MatmulPerfMode.DoubleRow

    Q = T // 4

    nc = bacc.Bacc("TRN2")
    img_d = nc.dram_tensor("img", [3, H, W], F32, kind="ExternalInput")
    sty_d = nc.dram_tensor("sty", [3, 128, W], F32, kind="ExternalInput")
    o_d = nc.dram_tensor("out", [2, 128, 128], F32, kind="ExternalOutput")

    img_v = [img_d[c, :, :].rearrange("(p r) w -> p (r w)", p=128) for c in range(3)]
    sty_v = [sty_d[c, :, :] for c in range(3)]

    with TileContext(nc) as tc:
        with ExitStack() as ctx:
            xpool = ctx.enter_context(tc.tile_pool(name="x", bufs=2))
            fpool = ctx.enter_context(tc.tile_pool(name="f", bufs=2))
            bpool = ctx.enter_context(tc.tile_pool(name="b", bufs=2))
            kpool = ctx.enter_context(tc.tile_pool(name="k", bufs=2))
            jpool = ctx.enter_context(tc.tile_pool(name="j", bufs=1))
            epool = ctx.enter_context(tc.tile_pool(name="e", bufs=2))
            cpool = ctx.enter_context(tc.tile_pool(name="c", bufs=1))
            opool = ctx.enter_context(tc.tile_pool(name="o", bufs=1))
            pspool = ctx.enter_context(tc.tile_pool(name="ps", bufs=2, space="PSUM"))

            # constants
            bcl1 = cpool.tile([128, 1], F32, tag="bcl1")
            nc.vector.memset(bcl1[:], 7.4)
            bcl2 = cpool.tile([128, 1], F32, tag="bcl2")
            nc.vector.memset(bcl2[:], 15.4)
            c56 = cpool.tile([128, 1], I32, tag="c56")
            nc.vector.memset(c56[:], 56)

            ps_img = pspool.tile([128, 128], F32)
            ps_sty = pspool.tile([128, 128], F32)

            def do_chunk(views, off, ps, start, stop, tcw=T):
                Qc = tcw // 4
                xt = xpool.tile([128, 3, T], F32, tag="xt")
                for c in range(3):
                    nc.sync.dma_start(xt[:, c, :tcw], views[c][:, off : off + tcw])

                # ACT: t = clamp(8x+8, 0, 15.4)  (u = Relu(-8x+7.4), in-place)
                tt = fpool.tile([128, 3, T], F32, tag="tt")
                for c in range(3):
                    nc.scalar.activation(tt[:, c, :tcw], xt[:, c, :tcw], Act.Relu,
                                         bias=bcl1[:], scale=-8.0)
                    nc.scalar.activation(tt[:, c, :tcw], tt[:, c, :tcw], Act.Relu,
                                         bias=bcl2[:], scale=-1.0)

                # ACT floors: r,g,rh,gh,h2=b (all i32)
                bins = bpool.tile([128, 2, T], I32, tag="bins")    # r, g
                bins4 = bpool.tile([128, 2, T], I32, tag="bins4")  # rh, gh
                h2 = kpool.tile([128, T], I32, tag="h2")           # b bin
                nc.scalar.activation(bins[:, 0, :tcw], tt[:, 0, :tcw], Act.Copy,
                                     bias=-0.5, scale=1.0)
                nc.scalar.activation(bins[:, 1, :tcw], tt[:, 1, :tcw], Act.Copy,
                                     bias=-0.5, scale=1.0)
                nc.scalar.activation(bins4[:, 0, :tcw], tt[:, 0, :tcw], Act.Copy,
                                     bias=-0.5, scale=0.25)
                nc.scalar.activation(bins4[:, 1, :tcw], tt[:, 1, :tcw], Act.Copy,
                                     bias=-0.5, scale=0.25)
                nc.scalar.activation(h2[:, :tcw], tt[:, 2, :tcw], Act.Copy,
                                     bias=-0.5, scale=1.0)

                r = bins[:, 0, :tcw]
                g = bins[:, 1, :tcw]
                rh = bins4[:, 0, :tcw]
                gh = bins4[:, 1, :tcw]

                # Pool (GPSIMD) integer chains: rl = r-4rh, gl = g-4gh,
                # h1 = rh+4gl  (all i32, via adds/subtract)
                pi = kpool.tile([128, 4, T], I32, tag="pi")
                # pi rows: 0=scratch 1=rl 2=gl 3=h1
                nc.gpsimd.tensor_tensor(pi[:, 0, :tcw], rh, rh, Alu.add)      # 2rh
                nc.gpsimd.tensor_tensor(pi[:, 0, :tcw], pi[:, 0, :tcw],
                                        pi[:, 0, :tcw], Alu.add)               # 4rh
                nc.gpsimd.tensor_tensor(pi[:, 1, :tcw], r, pi[:, 0, :tcw],
                                        Alu.subtract)                          # rl
                nc.gpsimd.tensor_tensor(pi[:, 0, :tcw], gh, gh, Alu.add)       # 2gh
                nc.gpsimd.tensor_tensor(pi[:, 0, :tcw], pi[:, 0, :tcw],
                                        pi[:, 0, :tcw], Alu.add)               # 4gh
                nc.gpsimd.tensor_tensor(pi[:, 2, :tcw], g, pi[:, 0, :tcw],
                                        Alu.subtract)                          # gl
                nc.gpsimd.tensor_tensor(pi[:, 0, :tcw], pi[:, 2, :tcw],
                                        pi[:, 2, :tcw], Alu.add)               # 2gl
                nc.gpsimd.tensor_tensor(pi[:, 0, :tcw], pi[:, 0, :tcw],
                                        pi[:, 0, :tcw], Alu.add)               # 4gl
                nc.gpsimd.tensor_tensor(pi[:, 3, :tcw], rh, pi[:, 0, :tcw],
                                        Alu.add)                               # h1

                rl = pi[:, 1, :tcw]
                gl = pi[:, 2, :tcw]
                h1 = pi[:, 3, :tcw]

                # t = 56 << (8*lo): s8 on ACT (Copy scale=8), shift on DVE
                s8 = jpool.tile([128, 2, T], I32, tag="s8")
                t1 = kpool.tile([128, T], I32, tag="t1")
                t2 = kpool.tile([128, T], I32, tag="t2")
                c56b = c56[:].broadcast_to([128, tcw])
                nc.scalar.activation(s8[:, 0, :tcw], rl, Act.Copy,
                                     bias=0.0, scale=8.0)
                nc.scalar.activation(s8[:, 1, :tcw], gh, Act.Copy,
                                     bias=0.0, scale=8.0)
                nc.vector.tensor_tensor(t1[:, :tcw], c56b, s8[:, 0, :tcw],
                                        Alu.logical_shift_left)
                nc.vector.tensor_tensor(t2[:, :tcw], c56b, s8[:, 1, :tcw],
                                        Alu.logical_shift_left)

                # DVE: packed planes, pixel t=(q,s,c)
                h1v = h1.rearrange("p (q s c) -> p q s c", s=2, c=2)
                t1v = t1[:, :tcw].rearrange("p (q s c) -> p q s c", s=2, c=2)
                e1p = epool.tile([128, Q, 2, 2, 16], I32, tag="e1p")
                e2c = epool.tile([128, 16, T], I32, tag="e2c")
                for e in range(16):
                    nc.vector.scalar_tensor_tensor(e1p[:, :Qc, :, :, e], h1v, e,
                                                   t1v, Alu.is_equal, Alu.mult)
                    nc.vector.scalar_tensor_tensor(e2c[:, e, :tcw], h2[:, :tcw],
                                                   e, t2[:, :tcw],
                                                   Alu.is_equal, Alu.mult)

                e1f = e1p[:, :, :, :, :].bitcast(FP8).rearrange(
                    "p q s c eb -> p q s (c eb)")
                e2f = e2c[:, :, :tcw].bitcast(FP8).rearrange(
                    "p e (q s cb) -> p q s e cb", s=2, cb=8)
                for q in range(Qc):
                    nc.tensor.matmul(ps[:], e1f[:, q, :, :], e2f[:, q, :, :, :],
                                     start=(start and q == 0),
                                     stop=(stop and q == Qc - 1),
                                     perf_mode=DR)

            img_sizes = [128, 256] + [512] * 15 + [128]
            assert sum(img_sizes) == IMG_PP
            off = 0
            for ci, tcw in enumerate(img_sizes):
                do_chunk(img_v, off, ps_img, ci == 0,
                         ci == len(img_sizes) - 1, tcw)
                off += tcw
            sty_sizes = [512, 384, 128]
            assert sum(sty_sizes) == STY_PP
            off = 0
            for ci, tcw in enumerate(sty_sizes):
                do_chunk(sty_v, off, ps_sty, ci == 0,
                         ci == len(sty_sizes) - 1, tcw)
                off += tcw

            ostage = opool.tile([128, 2, 128], F32)
            nc.vector.tensor_copy(ostage[:, 0, :], ps_img[:])
            nc.vector.tensor_copy(ostage[:, 1, :], ps_sty[:])
            nc.sync.dma_start(o_d[0, :, :], ostage[:, 0, :])
            nc.sync.dma_start(o_d[1, :, :], ostage[:, 1, :])

    nc.finalize()
    return nc


def _get_built():
    if "nc" not in _cache:
        _cache["nc"] = _build()
    return _cache["nc"]


def _perm():
    """flat[key1, key2] -> flat bin index (r + 16g + 256b)."""
    if "perm" in _cache:
        return _cache["perm"]
    k1 = np.arange(64)
    k2 = np.arange(64)
    rl = k1 & 3
    h1 = k1 >> 2
    rh = h1 & 3
    gl = h1 >> 2
    r = 4 * rh + rl            # [64]
    gh = k2 & 3
    b = k2 >> 2
    flat = (r[:, None] + 16 * (gl[:, None] + 4 * gh[None, :])
            + 256 * b[None, :])  # [64,64]
    _cache["perm"] = flat
    return flat


def _col_idx():
    if "colidx" not in _cache:
        k2 = np.arange(64)
        _cache["colidx"] = [(k2 // 4) * 8 + c * 4 + (k2 % 4) for c in range(2)]
    return _cache["colidx"]


def _decode(raw):
    """raw [2,128,128] f32 -> (hist_img[4096], hist_sty[4096]) exact counts."""
    flat = _perm()
    n0, n1 = _col_idx()
    out = []
    for s_ in range(2):
        m = raw[s_]
        counts64 = m[0:64, :][:, n0] + m[64:128, :][:, n1]   # [key1, key2]
        h = np.zeros(4096)
        np.add.at(h, flat.reshape(-1), counts64.reshape(-1))
        out.append(h)
    return out


def kernel(input, style_image, n_bins):
    assert int(n_bins) == 16
    from concourse import bass_utils

    nc = _get_built()
    input = np.ascontiguousarray(np.asarray(input, dtype=np.float32))
    style = np.ascontiguousarray(np.asarray(style_image, dtype=np.float32))
    B = input.shape[0]
    assert B == 8 and input.shape == (8, 3, H, W)
    in_maps = [
        {
            "img": input[i],
            "sty": np.ascontiguousarray(style[0, :, 128 * i : 128 * (i + 1), :]),
        }
        for i in range(8)
    ]
    res = bass_utils.run_bass_kernel_spmd(nc, in_maps, core_ids=list(range(8)),
                                          **_cache.get("run_kwargs", {}))
    _cache["last_results"] = res
    hists = np.zeros((B, 4096))
    sty_hist = np.zeros(4096)
    for i in range(8):
        hi, hs = _decode(res.results[i]["out"])
        hists[i] = hi
        sty_hist += hs
    cols = (hists / HW).astype(np.float32)
    target = (sty_hist / HW).astype(np.float32)
    loss = np.mean(np.abs(cols - target[None, :]).astype(np.float32))
    return np.float32(loss)
